# revision 17
# baseline (speedup 1.0000x reference)
"""GCN layer (message passing) on 8 Trainium2 NeuronCores.

Strategy (vertex-cut data parallelism):
  - dst nodes sharded across 8 cores (6250 each); edges partitioned by dst owner.
  - feature table replicated per core in fp16 (halved gather traffic);
    random-graph halo == whole graph, so replication == all-to-all, but cheaper.
  - Per core: dma_gather of src rows (int16 indices -> table split in two
    25000-row halves), one-hot fp16 matmuls aggregate edges into transposed
    agg [fin, dst] PSUM tiles (contraction over the 128-edge partition dim),
    then dense matmuls apply W / W_self / bias with no transposes anywhere.
  - All degree normalization folded into per-edge coefficients on the host:
      c_e = e_w[e] * outdeg^-1/2[src] * indeg^-1/2[dst]
    out[d] = feat[d] @ W_self + (sum_e c_e feat[src_e]) @ W + indeg^-1/2[d]*b
"""

import os
import sys

sys.path.insert(0, "/opt/trn_rl_repo")

import numpy as np

N = 50000
E = 800000
F = 256
NCORES = 8
PER = N // NCORES          # 6250 dst nodes per core
HALF = N // 2              # feature table split for int16 gather indices
WIN = 128                  # dst window (one PSUM tile column block)
NWIN = (PER + WIN - 1) // WIN   # 49 windows per core (last one 106 dsts)
SSW = 6                    # windows per superwindow (PSUM residency group)

_cache = {}


def _preprocess(feature, e_w, W_self, W, b, src, dst):
    f32 = np.float32
    src = np.asarray(src)
    dst = np.asarray(dst)
    ones = np.ones(E, f32)
    out_deg = np.maximum(np.bincount(src, weights=ones, minlength=N), 1.0)
    in_deg = np.maximum(np.bincount(dst, weights=ones, minlength=N), 1.0)
    odi = (out_deg ** -0.5).astype(f32)
    idi = (in_deg ** -0.5).astype(f32)
    c_all = (np.asarray(e_w)[:, 0].astype(f32) * odi[src] * idi[dst]).astype(f32)

    core = dst // PER
    dl = dst % PER
    w = dl // WIN
    dr = (dl % WIN).astype(f32)
    half = (src >= HALF).astype(np.int64)
    src_rel = (src - half * HALF).astype(np.int64)

    sws = [list(range(s, min(s + SSW, NWIN))) for s in range(0, NWIN, SSW)]
    sw_of = np.zeros(NWIN, np.int64)
    for si, wins in enumerate(sws):
        for wi in wins:
            sw_of[wi] = si
    nsw = len(sws)

    # run = (superwindow, half); edges ordered by window within a run
    rid = (core * nsw + sw_of[w]) * 2 + half
    nruns_g = NCORES * nsw * 2
    rcounts = np.bincount(rid, minlength=nruns_g).reshape(NCORES, nsw, 2)
    run_blocks = np.ceil(rcounts.max(axis=0) / 128).astype(np.int64)   # [nsw, 2]

    # within-run rank, ordered by (window, arbitrary): sort edges by composite key
    okey = rid * (NWIN + 1) + w
    order = np.argsort(okey, kind="stable")
    # rank within run
    rsorted = rid[order]
    rstart = np.searchsorted(rsorted, np.arange(nruns_g))
    rank_sorted = np.arange(E) - rstart[rsorted]
    rank = np.empty(E, np.int64)
    rank[order] = rank_sorted

    # run start offsets (in blocks) in emission order: sw-major, half inner
    b0r = np.zeros((nsw, 2), np.int64)
    runs = []
    nbt = 0
    for si in range(nsw):
        for h in (0, 1):
            b0r[si, h] = nbt
            nbt += int(run_blocks[si, h])
            runs.append((h, sws[si], int(run_blocks[si, h])))
    nb_tot = int(nbt)

    slot = b0r[sw_of[w], half] * 128 + rank

    idx16 = np.zeros((NCORES, nb_tot * 128), np.int16)
    wslot = np.full((NCORES, nb_tot * 128), -1, np.int64)   # window of each slot
    drl = np.zeros((NCORES, nb_tot * 128), f32)             # dl % WIN
    cw = np.zeros((NCORES, nb_tot * 128), f32)
    for k in range(NCORES):
        m = core == k
        idx16[k, slot[m]] = src_rel[m].astype(np.int16)
        wslot[k, slot[m]] = w[m]
        drl[k, slot[m]] = dr[m]
        cw[k, slot[m]] = c_all[m]

    # per block: global reference window w0 = min over cores of min window present;
    # union of windows present in the block across cores
    wsl = wslot.reshape(NCORES, nb_tot, 128)
    wmin = np.where(wsl < 0, NWIN + 1, wsl).min(axis=(0, 2))     # [nb_tot]
    wmax = wsl.max(axis=(0, 2))                                   # [nb_tot] (-1 if all pad)
    # fill pad-only blocks (possible at run tails) with the run's last window
    pos = 0
    blk_wins = []
    for i, (h, wins, rb) in enumerate(runs):
        for _bi in range(rb):
            lo_w, hi_w = int(wmin[pos]), int(wmax[pos])
            if hi_w < 0:            # all-pad block
                lo_w = hi_w = wins[-1]
            if lo_w > hi_w:
                lo_w = hi_w
            present = set()
            for k in range(NCORES):
                present.update(int(x) for x in np.unique(wsl[k, pos]) if x >= 0)
            if not present:
                present = {wins[-1]}
            blk_wins.append((lo_w, sorted(present)))
            pos += 1

    # bake per-core dst_rel relative to the block's global reference window
    w0_blk = np.array([bw[0] for bw in blk_wins], np.int64)       # [nb_tot]
    w0_slot = np.repeat(w0_blk, 128).reshape(1, -1)
    drel = np.where(wslot >= 0, (wslot - w0_slot) * WIN + drl, 0.0).astype(f32)
    max_j = max((bw[1][-1] - bw[0]) for bw in blk_wins)

    idx_t = np.tile(idx16.reshape(NCORES, -1, 16).transpose(0, 2, 1), (1, 8, 1))
    idx_t = np.ascontiguousarray(idx_t)                       # [NC,128,nbt*8]
    drel_t = np.ascontiguousarray(drel.reshape(NCORES, nb_tot, 128).transpose(0, 2, 1))
    cw_t = np.ascontiguousarray(cw.reshape(NCORES, nb_tot, 128).transpose(0, 2, 1))

    featf = np.asarray(feature, dtype=f32)
    tbl0 = featf[:HALF].astype(np.float16)
    tbl1 = featf[HALF:].astype(np.float16)
    featT = np.empty((NCORES, 128, 2, PER), np.float16)
    for k in range(NCORES):
        ft = featf[k * PER:(k + 1) * PER].T.astype(np.float16)   # [256, PER]
        featT[k] = ft.reshape(2, 128, PER).transpose(1, 0, 2)
    s16 = idi.astype(np.float16).reshape(NCORES, 1, PER)
    w16 = np.asarray(W, f32).astype(np.float16).reshape(2, 128, F).transpose(1, 0, 2)
    ws16 = np.asarray(W_self, f32).astype(np.float16).reshape(2, 128, F).transpose(1, 0, 2)
    b16 = np.asarray(b, f32).astype(np.float16).reshape(1, F)

    in_maps = []
    for k in range(NCORES):
        in_maps.append({
            "tbl0": tbl0, "tbl1": tbl1,
            "idx": idx_t[k], "drel": drel_t[k], "cw": cw_t[k],
            "featT": featT[k], "sinv": s16[k],
            "wm": np.ascontiguousarray(w16), "wself": np.ascontiguousarray(ws16),
            "bias": b16,
        })

    struct = (tuple((h, tuple(ws), rb) for h, ws, rb in runs),
              tuple((a, tuple(b)) for a, b in blk_wins), max_j)
    return in_maps, blk_wins, runs, nb_tot, max_j, struct


def _build(blk_wins, runs, nb_tot, max_j, reps=1, gmax=8, scratch=16384):
    import contextlib

    import concourse.bacc as bacc
    import concourse.mybir as mybir
    import concourse.tile as tile

    f16 = mybir.dt.float16
    f32 = mybir.dt.float32
    AOT = mybir.AluOpType
    GMAX = gmax

    nc = bacc.Bacc("TRN2", dynamic_dma_scratch_size=scratch)
    tbl = [nc.dram_tensor("tbl0", [HALF, F], f16, kind="ExternalInput"),
           nc.dram_tensor("tbl1", [HALF, F], f16, kind="ExternalInput")]
    idx_in = nc.dram_tensor("idx", [128, nb_tot * 8], mybir.dt.int16, kind="ExternalInput")
    drel_in = nc.dram_tensor("drel", [128, nb_tot], f32, kind="ExternalInput")
    cw_in = nc.dram_tensor("cw", [128, nb_tot], f32, kind="ExternalInput")
    featT_in = nc.dram_tensor("featT", [128, 2, PER], f16, kind="ExternalInput")
    s_in = nc.dram_tensor("sinv", [1, PER], f16, kind="ExternalInput")
    wm_in = nc.dram_tensor("wm", [128, 2, F], f16, kind="ExternalInput")
    ws_in = nc.dram_tensor("wself", [128, 2, F], f16, kind="ExternalInput")
    b_in = nc.dram_tensor("bias", [1, F], f16, kind="ExternalInput")
    out = nc.dram_tensor("out", [PER, F], f32, kind="ExternalOutput")

    NJ = max_j + 1

    with tile.TileContext(nc) as tc:
        with tc.tile_pool(name="const", bufs=1) as cpool, \
             tc.tile_pool(name="gp", bufs=6) as gpool, \
             tc.tile_pool(name="work", bufs=8) as wpool, \
             tc.tile_pool(name="hout", bufs=3) as hpool, \
             tc.tile_pool(name="psA", bufs=6, space="PSUM") as psA, \
             tc.tile_pool(name="psH", bufs=2, space="PSUM") as psH:

            iota_i = cpool.tile([128, NJ * WIN], mybir.dt.int16)
            nc.gpsimd.iota(iota_i[:], pattern=[[1, NJ * WIN]], base=0,
                           channel_multiplier=0)
            iota_f = cpool.tile([128, NJ * WIN], f16)
            nc.vector.tensor_copy(iota_f[:], iota_i[:])

            idx_t = cpool.tile([128, nb_tot * 8], mybir.dt.int16)
            nc.sync.dma_start(idx_t[:], idx_in[:])
            drel_t = cpool.tile([128, nb_tot], f32)
            nc.sync.dma_start(drel_t[:], drel_in[:])
            cw_t = cpool.tile([128, nb_tot], f32)
            nc.sync.dma_start(cw_t[:], cw_in[:])
            featT_t = cpool.tile([128, 2, PER], f16)
            nc.sync.dma_start(featT_t[:], featT_in[:])
            s_t = cpool.tile([128, PER], f16)
            nc.sync.dma_start(s_t[:1, :], s_in[:])
            wm_t = cpool.tile([128, 2, F], f16)
            nc.sync.dma_start(wm_t[:], wm_in[:])
            ws_t = cpool.tile([128, 2, F], f16)
            nc.sync.dma_start(ws_t[:], ws_in[:])
            b_t = cpool.tile([128, F], f16)
            nc.sync.dma_start(b_t[:1, :], b_in[:])

            def emit_all(rep):
                aggT = {}
                bank_of = {}
                pos = 0          # global block index
                ri = 0
                for swi in range(0, len(runs), 2):
                    wins = list(runs[swi][1])
                    banks = [psA.tile([128, 4 * WIN], f32, tag="aggT",
                                      name=f"aggT_{rep}_{swi}_{j}")
                             for j in range((2 * len(wins) + 3) // 4)]
                    for wl, wi in enumerate(wins):
                        for fh in (0, 1):
                            u = 2 * wl + fh
                            aggT[(wi, fh)] = banks[u // 4][:, (u % 4) * WIN:(u % 4 + 1) * WIN]
                            bank_of[(wi, fh)] = (swi, u // 4)
                    # pre-pass: matmul count per bank (both halves of this sw)
                    left = {}
                    started = {}
                    p0 = pos
                    for h in (0, 1):
                        rb = runs[swi + h][2]
                        for b in range(rb):
                            lo_w, present = blk_wins[p0]
                            for wi in present:
                                for fh in (0, 1):
                                    k = bank_of[(wi, fh)]
                                    left[k] = left.get(k, 0) + 1
                                    started.setdefault(k, False)
                            p0 += 1
                    # emission
                    touched = set()
                    for h in (0, 1):
                        hh, rwins, rb = runs[swi + h]
                        assert hh == h and list(rwins) == wins
                        if rb == 0:
                            continue
                        B0 = pos
                        gtiles = []
                        for c0 in range(0, rb, GMAX):
                            cb = min(GMAX, rb - c0)
                            g = gpool.tile([128, cb, F], f16, tag="g",
                                           name=f"g_{rep}_{swi}_{h}_{c0}")
                            nc.gpsimd.dma_gather(
                                g[:], tbl[h][:], idx_t[:, (B0 + c0) * 8:(B0 + c0 + cb) * 8],
                                cb * 128, cb * 128, F)
                            gtiles.append(g)
                        for b in range(rb):
                            B = B0 + b
                            lo_w, present = blk_wins[B]
                            g = gtiles[b // GMAX]
                            gi = b % GMAX
                            for wi in present:
                                j = wi - lo_w
                                oh = wpool.tile([128, WIN], f16, tag="oh",
                                                name=f"oh_{rep}_{B}_{j}")
                                nc.vector.tensor_scalar(
                                    out=oh[:], in0=iota_f[:, j * WIN:(j + 1) * WIN],
                                    scalar1=drel_t[:, B:B + 1], scalar2=cw_t[:, B:B + 1],
                                    op0=AOT.is_equal, op1=AOT.mult)
                                touched.add(wi)
                                for fh in (0, 1):
                                    k = bank_of[(wi, fh)]
                                    st = not started[k]
                                    started[k] = True
                                    left[k] -= 1
                                    nc.tensor.matmul(
                                        aggT[(wi, fh)],
                                        lhsT=g[:, gi, 128 * fh:128 * (fh + 1)],
                                        rhs=oh[:], start=st, stop=left[k] == 0,
                                        skip_group_check=True)
                        pos += rb
                    # dense phase for this superwindow
                    for wi in wins:
                        w0 = wi * WIN
                        wd = min(WIN, PER - w0)
                        h_ps = psH.tile([128, F], f32, tag="h", name=f"h_{rep}_{wi}")
                        first = True
                        if wi in touched:
                            for fh in (0, 1):
                                asb = wpool.tile([128, WIN], f16, tag=f"asb{fh}",
                                                 name=f"asb{fh}_{rep}_{wi}")
                                nc.scalar.copy(asb[:], aggT[(wi, fh)])
                                nc.tensor.matmul(h_ps[:wd, :], lhsT=asb[:, :wd],
                                                 rhs=wm_t[:, fh, :], start=first, stop=False)
                                first = False
                        for fh in (0, 1):
                            nc.tensor.matmul(h_ps[:wd, :], lhsT=featT_t[:, fh, w0:w0 + wd],
                                             rhs=ws_t[:, fh, :], start=first, stop=False)
                            first = False
                        nc.tensor.matmul(h_ps[:wd, :], lhsT=s_t[:1, w0:w0 + wd],
                                         rhs=b_t[:1, :], start=False, stop=True)
                        h_sb = hpool.tile([128, F], f32, tag="hsb", name=f"hsb_{rep}_{wi}")
                        nc.scalar.copy(h_sb[:wd, :], h_ps[:wd, :])
                        nc.sync.dma_start(out[w0:w0 + wd, :], h_sb[:wd, :])

            if reps > 1:
                with tc.For_i(0, reps, 1):
                    emit_all(0)
            else:
                emit_all(0)

    nc.compile()
    return nc


def kernel(feature, e_w, snorm_n, snorm_e, W_self, W, b, src, dst):
    from concourse.bass_utils import run_bass_kernel_spmd

    in_maps, blk_wins, runs, nb_tot, max_j, struct = _preprocess(
        feature, e_w, W_self, W, b, src, dst)

    nc = _cache.get(struct)
    if nc is None:
        nc = _build(blk_wins, runs, nb_tot, max_j)
        _cache[struct] = nc

    trace = bool(int(os.environ.get("KERNEL_TRACE", "0")))
    res = run_bass_kernel_spmd(nc, in_maps, core_ids=list(range(NCORES)),
                               trace=trace)
    kernel.last_results = res
    outp = np.concatenate([res.results[k]["out"] for k in range(NCORES)], axis=0)
    return outp, np.asarray(e_w)


# revision 19
# speedup vs baseline: 1.5818x; 1.5818x over previous
"""GCN layer (message passing) on 8 Trainium2 NeuronCores.

Strategy (vertex-cut data parallelism):
  - dst nodes sharded across 8 cores (6250 each); edges partitioned by dst owner.
  - feature table replicated per core in fp16 (halved gather traffic);
    random-graph halo == whole graph, so replication == all-to-all, but cheaper.
  - Per core: dma_gather of src rows (int16 indices -> table split in two
    25000-row halves), one-hot fp16 matmuls aggregate edges into transposed
    agg [fin, dst] PSUM tiles (contraction over the 128-edge partition dim),
    then dense matmuls apply W / W_self / bias with no transposes anywhere.
  - All degree normalization folded into per-edge coefficients on the host:
      c_e = e_w[e] * outdeg^-1/2[src] * indeg^-1/2[dst]
    out[d] = feat[d] @ W_self + (sum_e c_e feat[src_e]) @ W + indeg^-1/2[d]*b
"""

import os
import sys

sys.path.insert(0, "/opt/trn_rl_repo")

import numpy as np

N = 50000
E = 800000
F = 256
NCORES = 8
PER = N // NCORES          # 6250 dst nodes per core
HALF = N // 2              # feature table split for int16 gather indices
WIN = 128                  # dst window (one PSUM tile column block)
NWIN = (PER + WIN - 1) // WIN   # 49 windows per core (last one 106 dsts)
SSW = 6                    # windows per superwindow (PSUM residency group)

_cache = {}


def _preprocess(feature, e_w, W_self, W, b, src, dst):
    f32 = np.float32
    src = np.asarray(src)
    dst = np.asarray(dst)
    ones = np.ones(E, f32)
    out_deg = np.maximum(np.bincount(src, weights=ones, minlength=N), 1.0)
    in_deg = np.maximum(np.bincount(dst, weights=ones, minlength=N), 1.0)
    odi = (out_deg ** -0.5).astype(f32)
    idi = (in_deg ** -0.5).astype(f32)
    c_all = (np.asarray(e_w)[:, 0].astype(f32) * odi[src] * idi[dst]).astype(f32)

    core = dst // PER
    dl = dst % PER
    w = dl // WIN
    dr = (dl % WIN).astype(f32)
    half = (src >= HALF).astype(np.int64)
    src_rel = (src - half * HALF).astype(np.int64)

    sws = [list(range(s, min(s + SSW, NWIN))) for s in range(0, NWIN, SSW)]
    sw_of = np.zeros(NWIN, np.int64)
    for si, wins in enumerate(sws):
        for wi in wins:
            sw_of[wi] = si
    nsw = len(sws)

    # run = (superwindow, half); edges ordered by window within a run
    rid = (core * nsw + sw_of[w]) * 2 + half
    nruns_g = NCORES * nsw * 2
    rcounts = np.bincount(rid, minlength=nruns_g).reshape(NCORES, nsw, 2)
    run_blocks = np.ceil(rcounts.max(axis=0) / 128).astype(np.int64)   # [nsw, 2]

    # within-run rank, ordered by (window, arbitrary): sort edges by composite key
    okey = rid * (NWIN + 1) + w
    order = np.argsort(okey, kind="stable")
    # rank within run
    rsorted = rid[order]
    rstart = np.searchsorted(rsorted, np.arange(nruns_g))
    rank_sorted = np.arange(E) - rstart[rsorted]
    rank = np.empty(E, np.int64)
    rank[order] = rank_sorted

    # run start offsets (in blocks) in emission order: sw-major, half inner
    b0r = np.zeros((nsw, 2), np.int64)
    runs = []
    nbt = 0
    for si in range(nsw):
        for h in (0, 1):
            b0r[si, h] = nbt
            nbt += int(run_blocks[si, h])
            runs.append((h, sws[si], int(run_blocks[si, h])))
    nb_tot = int(nbt)

    slot = b0r[sw_of[w], half] * 128 + rank

    idx16 = np.zeros((NCORES, nb_tot * 128), np.int16)
    wslot = np.full((NCORES, nb_tot * 128), -1, np.int64)   # window of each slot
    drl = np.zeros((NCORES, nb_tot * 128), f32)             # dl % WIN
    cw = np.zeros((NCORES, nb_tot * 128), f32)
    for k in range(NCORES):
        m = core == k
        idx16[k, slot[m]] = src_rel[m].astype(np.int16)
        wslot[k, slot[m]] = w[m]
        drl[k, slot[m]] = dr[m]
        cw[k, slot[m]] = c_all[m]

    # per block: global reference window w0 = min over cores of min window present;
    # union of windows present in the block across cores
    wsl = wslot.reshape(NCORES, nb_tot, 128)
    wmin = np.where(wsl < 0, NWIN + 1, wsl).min(axis=(0, 2))     # [nb_tot]
    wmax = wsl.max(axis=(0, 2))                                   # [nb_tot] (-1 if all pad)
    # fill pad-only blocks (possible at run tails) with the run's last window
    pos = 0
    blk_wins = []
    for i, (h, wins, rb) in enumerate(runs):
        for _bi in range(rb):
            lo_w, hi_w = int(wmin[pos]), int(wmax[pos])
            if hi_w < 0:            # all-pad block
                lo_w = hi_w = wins[-1]
            if lo_w > hi_w:
                lo_w = hi_w
            present = set()
            for k in range(NCORES):
                present.update(int(x) for x in np.unique(wsl[k, pos]) if x >= 0)
            if not present:
                present = {wins[-1]}
            blk_wins.append((lo_w, sorted(present)))
            pos += 1

    # bake per-core dst_rel relative to the block's global reference window
    w0_blk = np.array([bw[0] for bw in blk_wins], np.int64)       # [nb_tot]
    w0_slot = np.repeat(w0_blk, 128).reshape(1, -1)
    drel = np.where(wslot >= 0, (wslot - w0_slot) * WIN + drl, 0.0).astype(f32)
    max_j = max((bw[1][-1] - bw[0]) for bw in blk_wins)

    idx_t = np.tile(idx16.reshape(NCORES, -1, 16).transpose(0, 2, 1), (1, 8, 1))
    idx_t = np.ascontiguousarray(idx_t)                       # [NC,128,nbt*8]
    drel_t = np.ascontiguousarray(drel.reshape(NCORES, nb_tot, 128).transpose(0, 2, 1))
    cw_t = np.ascontiguousarray(cw.reshape(NCORES, nb_tot, 128).transpose(0, 2, 1))

    featf = np.asarray(feature, dtype=f32)
    tbl0 = featf[:HALF].astype(np.float16)
    tbl1 = featf[HALF:].astype(np.float16)
    featT = np.empty((NCORES, 128, 2, PER), np.float16)
    for k in range(NCORES):
        ft = featf[k * PER:(k + 1) * PER].T.astype(np.float16)   # [256, PER]
        featT[k] = ft.reshape(2, 128, PER).transpose(1, 0, 2)
    s16 = idi.astype(np.float16).reshape(NCORES, 1, PER)
    w16 = np.asarray(W, f32).astype(np.float16).reshape(2, 128, F).transpose(1, 0, 2)
    ws16 = np.asarray(W_self, f32).astype(np.float16).reshape(2, 128, F).transpose(1, 0, 2)
    b16 = np.asarray(b, f32).astype(np.float16).reshape(1, F)

    in_maps = []
    for k in range(NCORES):
        in_maps.append({
            "tbl0": tbl0, "tbl1": tbl1,
            "idx": idx_t[k], "drel": drel_t[k], "cw": cw_t[k],
            "featT": featT[k], "sinv": s16[k],
            "wm": np.ascontiguousarray(w16), "wself": np.ascontiguousarray(ws16),
            "bias": b16,
        })

    struct = (tuple((h, tuple(ws), rb) for h, ws, rb in runs),
              tuple((a, tuple(b)) for a, b in blk_wins), max_j)
    return in_maps, blk_wins, runs, nb_tot, max_j, struct


def _build(blk_wins, runs, nb_tot, max_j, reps=1, gmax=8, scratch=16384, nq=4):
    import contextlib

    import concourse.bacc as bacc
    import concourse.mybir as mybir
    import concourse.tile as tile

    f16 = mybir.dt.float16
    f32 = mybir.dt.float32
    AOT = mybir.AluOpType
    GMAX = gmax

    nc = bacc.Bacc("TRN2", dynamic_dma_scratch_size=scratch, num_swdge_queues=nq)
    tbl = [nc.dram_tensor("tbl0", [HALF, F], f16, kind="ExternalInput"),
           nc.dram_tensor("tbl1", [HALF, F], f16, kind="ExternalInput")]
    idx_in = nc.dram_tensor("idx", [128, nb_tot * 8], mybir.dt.int16, kind="ExternalInput")
    drel_in = nc.dram_tensor("drel", [128, nb_tot], f32, kind="ExternalInput")
    cw_in = nc.dram_tensor("cw", [128, nb_tot], f32, kind="ExternalInput")
    featT_in = nc.dram_tensor("featT", [128, 2, PER], f16, kind="ExternalInput")
    s_in = nc.dram_tensor("sinv", [1, PER], f16, kind="ExternalInput")
    wm_in = nc.dram_tensor("wm", [128, 2, F], f16, kind="ExternalInput")
    ws_in = nc.dram_tensor("wself", [128, 2, F], f16, kind="ExternalInput")
    b_in = nc.dram_tensor("bias", [1, F], f16, kind="ExternalInput")
    out = nc.dram_tensor("out", [PER, F], f32, kind="ExternalOutput")

    NJ = max_j + 1

    with tile.TileContext(nc) as tc:
        with tc.tile_pool(name="const", bufs=1) as cpool, \
             tc.tile_pool(name="gp", bufs=6) as gpool, \
             tc.tile_pool(name="work", bufs=8) as wpool, \
             tc.tile_pool(name="hout", bufs=3) as hpool, \
             tc.tile_pool(name="psA", bufs=6, space="PSUM") as psA, \
             tc.tile_pool(name="psH", bufs=2, space="PSUM") as psH:

            iota_i = cpool.tile([128, NJ * WIN], mybir.dt.int16)
            nc.gpsimd.iota(iota_i[:], pattern=[[1, NJ * WIN]], base=0,
                           channel_multiplier=0)
            iota_f = cpool.tile([128, NJ * WIN], f16)
            nc.vector.tensor_copy(iota_f[:], iota_i[:])

            idx_t = cpool.tile([128, nb_tot * 8], mybir.dt.int16)
            nc.sync.dma_start(idx_t[:], idx_in[:])
            drel_t = cpool.tile([128, nb_tot], f32)
            nc.sync.dma_start(drel_t[:], drel_in[:])
            cw_t = cpool.tile([128, nb_tot], f32)
            nc.sync.dma_start(cw_t[:], cw_in[:])
            featT_t = cpool.tile([128, 2, PER], f16)
            nc.sync.dma_start(featT_t[:], featT_in[:])
            s_t = cpool.tile([128, PER], f16)
            nc.sync.dma_start(s_t[:1, :], s_in[:])
            wm_t = cpool.tile([128, 2, F], f16)
            nc.sync.dma_start(wm_t[:], wm_in[:])
            ws_t = cpool.tile([128, 2, F], f16)
            nc.sync.dma_start(ws_t[:], ws_in[:])
            b_t = cpool.tile([128, F], f16)
            nc.sync.dma_start(b_t[:1, :], b_in[:])

            def emit_all(rep):
                aggT = {}
                bank_of = {}
                pos = 0          # global block index
                ri = 0
                for swi in range(0, len(runs), 2):
                    wins = list(runs[swi][1])
                    banks = [psA.tile([128, 4 * WIN], f32, tag="aggT",
                                      name=f"aggT_{rep}_{swi}_{j}")
                             for j in range((2 * len(wins) + 3) // 4)]
                    for wl, wi in enumerate(wins):
                        for fh in (0, 1):
                            u = 2 * wl + fh
                            aggT[(wi, fh)] = banks[u // 4][:, (u % 4) * WIN:(u % 4 + 1) * WIN]
                            bank_of[(wi, fh)] = (swi, u // 4)
                    # pre-pass: matmul count per bank (both halves of this sw)
                    left = {}
                    started = {}
                    p0 = pos
                    for h in (0, 1):
                        rb = runs[swi + h][2]
                        for b in range(rb):
                            lo_w, present = blk_wins[p0]
                            for wi in present:
                                for fh in (0, 1):
                                    k = bank_of[(wi, fh)]
                                    left[k] = left.get(k, 0) + 1
                                    started.setdefault(k, False)
                            p0 += 1
                    # emission
                    touched = set()
                    for h in (0, 1):
                        hh, rwins, rb = runs[swi + h]
                        assert hh == h and list(rwins) == wins
                        if rb == 0:
                            continue
                        B0 = pos
                        gtiles = []
                        for c0 in range(0, rb, GMAX):
                            cb = min(GMAX, rb - c0)
                            g = gpool.tile([128, cb, F], f16, tag="g",
                                           name=f"g_{rep}_{swi}_{h}_{c0}")
                            nc.gpsimd.dma_gather(
                                g[:], tbl[h][:], idx_t[:, (B0 + c0) * 8:(B0 + c0 + cb) * 8],
                                cb * 128, cb * 128, F,
                                queue_num=(c0 // GMAX) % nq)
                            gtiles.append(g)
                        for b in range(rb):
                            B = B0 + b
                            lo_w, present = blk_wins[B]
                            g = gtiles[b // GMAX]
                            gi = b % GMAX
                            for wi in present:
                                j = wi - lo_w
                                oh = wpool.tile([128, WIN], f16, tag="oh",
                                                name=f"oh_{rep}_{B}_{j}")
                                nc.vector.tensor_scalar(
                                    out=oh[:], in0=iota_f[:, j * WIN:(j + 1) * WIN],
                                    scalar1=drel_t[:, B:B + 1], scalar2=cw_t[:, B:B + 1],
                                    op0=AOT.is_equal, op1=AOT.mult)
                                touched.add(wi)
                                for fh in (0, 1):
                                    k = bank_of[(wi, fh)]
                                    st = not started[k]
                                    started[k] = True
                                    left[k] -= 1
                                    nc.tensor.matmul(
                                        aggT[(wi, fh)],
                                        lhsT=g[:, gi, 128 * fh:128 * (fh + 1)],
                                        rhs=oh[:], start=st, stop=left[k] == 0,
                                        skip_group_check=True)
                        pos += rb
                    # dense phase for this superwindow
                    for wi in wins:
                        w0 = wi * WIN
                        wd = min(WIN, PER - w0)
                        h_ps = psH.tile([128, F], f32, tag="h", name=f"h_{rep}_{wi}")
                        first = True
                        if wi in touched:
                            for fh in (0, 1):
                                asb = wpool.tile([128, WIN], f16, tag=f"asb{fh}",
                                                 name=f"asb{fh}_{rep}_{wi}")
                                nc.scalar.copy(asb[:], aggT[(wi, fh)])
                                nc.tensor.matmul(h_ps[:wd, :], lhsT=asb[:, :wd],
                                                 rhs=wm_t[:, fh, :], start=first, stop=False)
                                first = False
                        for fh in (0, 1):
                            nc.tensor.matmul(h_ps[:wd, :], lhsT=featT_t[:, fh, w0:w0 + wd],
                                             rhs=ws_t[:, fh, :], start=first, stop=False)
                            first = False
                        nc.tensor.matmul(h_ps[:wd, :], lhsT=s_t[:1, w0:w0 + wd],
                                         rhs=b_t[:1, :], start=False, stop=True)
                        h_sb = hpool.tile([128, F], f32, tag="hsb", name=f"hsb_{rep}_{wi}")
                        nc.scalar.copy(h_sb[:wd, :], h_ps[:wd, :])
                        nc.sync.dma_start(out[w0:w0 + wd, :], h_sb[:wd, :])

            if reps > 1:
                with tc.For_i(0, reps, 1):
                    emit_all(0)
            else:
                emit_all(0)

    nc.compile()
    return nc


def kernel(feature, e_w, snorm_n, snorm_e, W_self, W, b, src, dst):
    from concourse.bass_utils import run_bass_kernel_spmd

    in_maps, blk_wins, runs, nb_tot, max_j, struct = _preprocess(
        feature, e_w, W_self, W, b, src, dst)

    nc = _cache.get(struct)
    if nc is None:
        nc = _build(blk_wins, runs, nb_tot, max_j)
        _cache[struct] = nc

    trace = bool(int(os.environ.get("KERNEL_TRACE", "0")))
    res = run_bass_kernel_spmd(nc, in_maps, core_ids=list(range(NCORES)),
                               trace=trace)
    kernel.last_results = res
    outp = np.concatenate([res.results[k]["out"] for k in range(NCORES)], axis=0)
    return outp, np.asarray(e_w)
